# revision 29
# baseline (speedup 1.0000x reference)
"""Trainium2 Bass kernel for causal cosine-sim attention block (8 cores), v2.

Reference computation:
  x [2, 2048, 1024] fp32
  xn = LayerNorm(x) * ln_w + ln_b
  qkv = xn @ W_qkv  -> q, k, v   (16 heads x 64)
  q, k l2-normalized per head-dim; sim = (q.k) * 8, causal mask, softmax
  o = attn @ v ; out = o @ W_out   [2, 2048, 1024] fp32

Sharding (8 cores): head-parallel QKV+attention (core c owns heads 2c,2c+1),
token-parallel out projection after per-batch AllToAll.

v2 design (vs v1's 460us):
  - QKV matmuls in flipped orientation psum[cols, tokens]: weights are the
    stationary operand, so each (batch, m-group, half) pass issues only
    8 LDWEIGHTS and streams 1024-token blocks.  The outputs land directly
    in the q^T/k^T [dhead, token] layout attention needs -- no x or qk
    DMA transposes at all.
  - LN mean correction as a K=8 matmul: psum += negcs (x) mu using a
    block-diagonal mu tile (built from a padded [128,128] DMA transpose
    of the per-token means).
  - q/k squared norms via a ones-block matmul (PE) instead of DVE reduce.
  - rstd folds into V after small [64,128] DMA transposes of v^T.
  - LN stats: batch 0 on DVE (bn_stats), batch 1 on ACT via
    activation(..., accum_out=) Copy/Square passes, so neither engine's
    in-order queue stalls the QKV evacuation chains.
  - Attention inner loop software-pipelined: S^T(kt+1) issued before
    PV(kt), so the ACT exp latency never idles the PE (keeps the PE
    p-state ramp at full clock).
  - AllToAll for batch 0 issued before batch 1's attention; both
    out-projections follow all attention matmuls.
  - softmax normalization reads the PV psum directly (no oU evac copy).
"""

import numpy as np

import concourse.bass as bass
import concourse.mybir as mybir
import concourse.tile as tile
from concourse import bacc
from concourse.bass import ts, ds

F32 = mybir.dt.float32
BF16 = mybir.dt.bfloat16

NCORES = 8
DIM = 1024
HEADS = 16
DHEAD = 64
INNER = HEADS * DHEAD          # 1024
B = 2
N = 2048
NTOK = B * N                   # 4096
TOK_HALF = N // NCORES         # 256 tokens per core per batch
HLOC = HEADS // NCORES         # 2 heads per core
P = 128
KT = N // P                    # 16 token tiles per batch
QB = N // 512                  # 4 q-blocks (512) per batch
EPS = 1e-5
SCALE = 8.0
DEBUG_DUMP = False
_DBG = {}
AluOp = mybir.AluOpType
Act = mybir.ActivationFunctionType


def build_kernel():
    nc = bacc.Bacc("TRN2", target_bir_lowering=False, debug=False,
                   num_devices=NCORES)

    x_t = nc.dram_tensor("x_t", [NTOK, DIM], BF16, kind="ExternalInput")
    x_T = nc.dram_tensor("x_T", [DIM, NTOK], BF16, kind="ExternalInput")
    w_flip = nc.dram_tensor("w_flip", [P, 3, 8, P], BF16,
                            kind="ExternalInput")
    negcs8 = nc.dram_tensor("negcs8", [8, 3, P], BF16, kind="ExternalInput")
    ones2 = nc.dram_tensor("ones2", [P, 2], BF16, kind="ExternalInput")
    eye = nc.dram_tensor("eye", [P, P], F32, kind="ExternalInput")
    w_out = nc.dram_tensor("w_out", [INNER, DIM], BF16, kind="ExternalInput")
    y_out = nc.dram_tensor("y_out", [B, TOK_HALF, DIM], F32,
                           kind="ExternalOutput")
    if DEBUG_DUMP:
        global _DBG
        _DBG = {
            "dbg_mu": nc.dram_tensor("dbg_mu", [P, P], BF16,
                                     kind="ExternalOutput"),
            "dbg_mudiag": nc.dram_tensor("dbg_mudiag", [4, B, 2, 2, 512],
                                         BF16, kind="ExternalOutput"),
            "dbg_rstd": nc.dram_tensor("dbg_rstd", [P, B, KT], F32,
                                       kind="ExternalOutput"),
            "dbg_qkT": nc.dram_tensor("dbg_qkT", [P, 2, B, N], BF16,
                                      kind="ExternalOutput"),
            "dbg_vsb": nc.dram_tensor("dbg_vsb", [P, B, KT, HLOC, DHEAD + 1],
                                      BF16, kind="ExternalOutput"),
            "dbg_oT": nc.dram_tensor("dbg_oT", [P, B, N], BF16,
                                     kind="ExternalOutput"),
            "dbg_xT": nc.dram_tensor("dbg_xT", [P, 8, NTOK], BF16,
                                     kind="ExternalOutput"),
            "dbg_qks": nc.dram_tensor("dbg_qks", [P, 1, N], BF16,
                                      kind="ExternalOutput"),
        }

    with tile.TileContext(nc) as tc:
        _body(nc, tc, x_t, x_T, w_flip, negcs8, ones2, eye, w_out, y_out)
    nc.compile()
    return nc


def _body(nc, tc, x_t, x_T, w_flip, negcs8, ones2, eye, w_out, y_out):
    import contextlib
    ctx = contextlib.ExitStack()
    with ctx:
        persist = ctx.enter_context(tc.tile_pool(name="persist", bufs=1))
        xrow_pool = ctx.enter_context(tc.tile_pool(name="xrow", bufs=2))
        small = ctx.enter_context(tc.tile_pool(name="small", bufs=2))
        evac = ctx.enter_context(tc.tile_pool(name="evac", bufs=2))
        vtmp_pool = ctx.enter_context(tc.tile_pool(name="vtmp", bufs=1))
        e_pool = ctx.enter_context(tc.tile_pool(name="epool", bufs=2))
        norm_pool = ctx.enter_context(tc.tile_pool(name="norm", bufs=2))
        out_pool = ctx.enter_context(tc.tile_pool(name="outp", bufs=2))
        dram = ctx.enter_context(tc.tile_pool(name="dram", bufs=1,
                                              space="DRAM"))

        # ---- persistent SBUF (per-partition bytes noted) ----
        xT_sb = persist.tile([P, 8, NTOK], BF16)            # 64 KB
        w_flip_sb = persist.tile([P, 3, 8, P], BF16)        # 6 KB
        negcs_sb = persist.tile([8, 3, P], BF16)
        ones2_sb = persist.tile([P, 2], BF16)
        eye_sb = persist.tile([P, P], F32)
        w_out_sb = persist.tile([P, 8, DIM], BF16)          # 16 KB
        qk_stage = persist.tile([P, 1, N], BF16)            # 4 KB
        qkT = persist.tile([P, 2, B, N], BF16)              # 16 KB
        v_sb = persist.tile([P, B, KT, HLOC, DHEAD + 1], BF16)  # 8.3 KB
        oT = persist.tile([P, B, N], BF16)                  # 8 KB
        oT_all = persist.tile([P, 8, B, TOK_HALF], BF16)    # 8 KB
        mu_all = persist.tile([P, P], BF16)                 # col = bi*16+ti
        muT = {}
        for bi in range(B):
            for half in range(2):
                muT[(bi, half)] = persist.tile([P, P], BF16,
                                               name=f"muT{bi}{half}")
        mudiag = persist.tile([4, B, 2, 2, 512], BF16)      # 8 KB
        mu_f = persist.tile([P, B, KT], F32)
        msq_all = persist.tile([P, B, KT], F32)
        var_all = persist.tile([P, B, KT], F32)
        rstd_all = persist.tile([P, B, KT], F32)
        scr = persist.tile([P, DIM], BF16)                  # ACT scratch
        rcpk = persist.tile([P, B, KT, HLOC], F32)          # 8/|k| columns
        oU_all = persist.tile([DHEAD + 1, B, QB, HLOC, 512], BF16)  # 8KB
        pack_o = persist.tile([8, B, 512], BF16)
        pack_q = persist.tile([8, B, 512], F32)
        eps_t = persist.tile([P, 1], F32)

        nc.scalar.dma_start(w_flip_sb[:], w_flip.ap())
        nc.scalar.dma_start(
            w_out_sb[:], w_out.ap().rearrange("(o p) c -> p o c", p=P))
        nc.sync.dma_start(negcs_sb[:], negcs8.ap())
        nc.sync.dma_start(ones2_sb[:], ones2.ap())
        nc.sync.dma_start(eye_sb[:], eye.ap())
        nc.vector.memset(eps_t[:], EPS)
        nc.vector.memset(mu_all[:], 0.0)
        nc.vector.memset(mudiag[:], 0.0)
        nc.vector.memset(v_sb[:, :, :, :, DHEAD], 1.0)

        # x^T loads on the scalar queue, batch 0 first
        xTr = x_T.ap().rearrange("(o p) t -> p o t", p=P)
        for bi in range(B):
            for k in range(8):
                nc.scalar.dma_start(xT_sb[:, k, ds(bi * N, N)],
                                    xTr[:, k, ds(bi * N, N)])

        cc_in = []
        cc_out = []
        for bi in range(B):
            cci = dram.tile([NCORES, P, TOK_HALF], BF16, name=f"cci{bi}")
            cco = dram.tile([NCORES, P, TOK_HALF], BF16, name=f"cco{bi}")
            cc_in.append(cci)
            cc_out.append(cco)

        # ================= Phase A: LN stats =================
        def stats_tail(bi, ti):
            # mu (bf16, for the transpose) and var = msq - mu^2
            nc.vector.tensor_copy(mu_all[:, bi * KT + ti:bi * KT + ti + 1],
                                  mu_f[:, bi, ti:ti + 1])
            musq = small.tile([P, 1], F32, tag="musq")
            nc.vector.tensor_tensor(musq[:], mu_f[:, bi, ti:ti + 1],
                                    mu_f[:, bi, ti:ti + 1], AluOp.mult)
            nc.vector.tensor_tensor(var_all[:, bi, ti:ti + 1],
                                    msq_all[:, bi, ti:ti + 1], musq[:],
                                    AluOp.subtract)

        def mu_transpose(bi, half):
            t = muT[(bi, half)]
            nc.sync.dma_start_transpose(t[:], mu_all[:])
            for b2 in range(2):
                for r in range(4):
                    row = bi * KT + half * 8 + b2 * 4 + r
                    nc.sync.dma_start(
                        mudiag[r:r + 1, bi, half, b2, ts(r, P)],
                        t[row:row + 1, 0:P])

        for bi in range(B):
            for ti in range(KT):
                i = bi * KT + ti
                xt = xrow_pool.tile([P, DIM], BF16, tag="xt")
                nc.sync.dma_start(xt[:], x_t.ap()[ts(i, P), :])
                if bi == 0:
                    # DVE path: bn_stats (frees ACT for DMA inits)
                    stats = small.tile([P, 2, 6], F32, tag="stats")
                    nc.vector.bn_stats(stats[:, 0, :], xt[:, 0:512])
                    nc.vector.bn_stats(stats[:, 1, :], xt[:, 512:1024])
                    mv = small.tile([P, 2], F32, tag="mv")
                    nc.vector.bn_aggr(mv[:], stats[:])
                    nc.vector.tensor_copy(var_all[:, bi, ti:ti + 1],
                                          mv[:, 1:2])
                    nc.vector.tensor_copy(
                        mu_all[:, i:i + 1], mv[:, 0:1])
                else:
                    # ACT path: accum_out row sums (runs under phase B PE)
                    nc.scalar.activation(scr[:], xt[:], Act.Copy,
                                         scale=1.0 / DIM,
                                         accum_out=mu_f[:, bi, ti:ti + 1])
                    nc.scalar.activation(scr[:], xt[:], Act.Square,
                                         scale=(1.0 / DIM) ** 0.5,
                                         accum_out=msq_all[:, bi, ti:ti + 1])
                    stats_tail(bi, ti)
                if ti == 7:
                    mu_transpose(bi, 0)
                elif ti == KT - 1:
                    mu_transpose(bi, 1)
            # rstd for this batch: one Sqrt + reciprocal
            sd = small.tile([P, KT], F32, tag="sd")
            nc.scalar.activation(sd[:], var_all[:, bi, :], Act.Sqrt,
                                 bias=eps_t[:])
            nc.vector.reciprocal(rstd_all[:, bi, :], sd[:])

        # ================= Phase B: QKV (flipped) =================
        psB = contextlib.ExitStack()
        ps_qkv_pool = psB.enter_context(
            tc.tile_pool(name="qkvps", bufs=2, space="PSUM"))
        ps_ssq_pool = psB.enter_context(
            tc.tile_pool(name="ssqps", bufs=2, space="PSUM"))
        ps_vt_pool = psB.enter_context(
            tc.tile_pool(name="vtps", bufs=2, space="PSUM"))

        for bi in range(B):
            for mg in (2, 0, 1):
                for half in range(2):
                    tok0 = bi * N + half * 1024
                    ps2 = [ps_qkv_pool.tile([P, 512], F32, tag=f"qkv{b2}",
                                            name=f"qkv{b2}")
                           for b2 in range(2)]
                    for k in range(8):
                        for b2 in range(2):
                            nc.tensor.matmul(
                                ps2[b2][:], lhsT=w_flip_sb[:, mg, k, :],
                                rhs=xT_sb[:, k, ds(tok0 + b2 * 512, 512)],
                                start=(k == 0), stop=False)
                    for b2 in range(2):
                        nc.tensor.matmul(
                            ps2[b2][:], lhsT=negcs_sb[0:4, mg, :],
                            rhs=mudiag[:, bi, half, b2, :],
                            start=False, stop=True)
                    if mg == 2:
                        vtmp = vtmp_pool.tile([P, 1024], F32, tag="vtmp")
                        for b2 in range(2):
                            nc.vector.tensor_copy(
                                vtmp[:, ds(b2 * 512, 512)], ps2[b2][:])
                        for c in range(8):
                            kt = half * 8 + c
                            ps_vt = ps_vt_pool.tile([P, P], F32, tag="vT")
                            nc.tensor.transpose(ps_vt[:], vtmp[:, ts(c, P)],
                                                eye_sb[:])
                            nc.vector.tensor_scalar_mul(
                                v_sb[:, bi, kt, :, 0:DHEAD],
                                ps_vt.rearrange("p (h d) -> p h d", d=DHEAD),
                                rstd_all[:, bi, kt:kt + 1])
                    elif mg == 1:
                        # k: store RAW k; 8/|k| folds into the exp scale
                        for b2 in range(2):
                            col = half * 1024 + b2 * 512
                            nc.vector.tensor_copy(
                                qkT[:, mg, bi, ds(col, 512)], ps2[b2][:])
                            seg = qkT[:, mg, bi, ds(col, 512)]
                            sqt = evac.tile([P, 512], BF16, tag="sqt")
                            nc.vector.tensor_tensor(sqt[:], seg, seg,
                                                    AluOp.mult)
                            ssq = ps_ssq_pool.tile([2, 512], F32,
                                                   tag="ssq")
                            nc.tensor.matmul(ssq[:], lhsT=ones2_sb[:],
                                             rhs=sqt[:], start=True,
                                             stop=True)
                            # |k|/8 rows, then PE-transpose to columns
                            sk = small.tile([2, 512], F32, tag="sk")
                            nc.scalar.activation(sk[:], ssq[:], Act.Sqrt,
                                                 scale=1.0 / 64.0)
                            for cc in range(4):
                                kt = half * 8 + b2 * 4 + cc
                                ps_kt = ps_vt_pool.tile([P, P], F32,
                                                        tag="vT",
                                                        name="pskt")
                                nc.tensor.transpose(
                                    ps_kt[:, 0:2], sk[:, ts(cc, P)],
                                    eye_sb[0:2, 0:2])
                                nc.vector.tensor_copy(
                                    rcpk[:, bi, kt, :], ps_kt[:, 0:2])
                        if half == 1:
                            # batched reciprocal of all |k|/8 for this batch
                            nc.vector.reciprocal(
                                rcpk[:, bi].rearrange("p t h -> p (t h)"),
                                rcpk[:, bi].rearrange("p t h -> p (t h)"))
                    else:
                        for b2 in range(2):
                            nc.vector.tensor_copy(
                                qk_stage[:, 0,
                                         ds(half * 1024 + b2 * 512, 512)],
                                ps2[b2][:])
                        for b2 in range(2):
                            col = half * 1024 + b2 * 512
                            seg = qk_stage[:, 0, ds(col, 512)]
                            sqt = evac.tile([P, 512], BF16, tag="sqt")
                            nc.vector.tensor_tensor(sqt[:], seg, seg,
                                                    AluOp.mult)
                            ssq = ps_ssq_pool.tile([2, 512], F32,
                                                   tag="ssq")
                            nc.tensor.matmul(ssq[:], lhsT=ones2_sb[:],
                                             rhs=sqt[:], start=True,
                                             stop=True)
                            rcp = small.tile([2, 512], F32, tag="rcp")
                            nc.scalar.activation(rcp[:], ssq[:], Act.Sqrt)
                            r0 = (half * 2 + b2) * 2
                            nc.sync.dma_start(pack_q[r0:r0 + 1, bi, :],
                                              rcp[0:1, :])
                            nc.sync.dma_start(pack_q[r0 + 1:r0 + 2, bi, :],
                                              rcp[1:2, :])
            # ---- batched q normalization for this batch ----
            nc.vector.reciprocal(pack_q[:, bi, :], pack_q[:, bi, :])
            for half in range(2):
                for b2 in range(2):
                    col = half * 1024 + b2 * 512
                    r0 = (half * 2 + b2) * 2
                    for h in range(2):
                        rowq = small.tile([1, 512], F32, tag="rowB")
                        nc.sync.dma_start(rowq[:],
                                          pack_q[r0 + h:r0 + h + 1, bi, :])
                        qbc = norm_pool.tile([DHEAD, 512], F32, tag="qbc")
                        nc.gpsimd.partition_broadcast(qbc[:], rowq[:])
                        if h == 0:
                            nc.vector.tensor_tensor(
                                qkT[0:DHEAD, 0, bi, ds(col, 512)],
                                qk_stage[0:DHEAD, 0, ds(col, 512)],
                                qbc[:], AluOp.mult)
                        else:
                            h1s = evac.tile([DHEAD, 512], BF16, tag="h1s")
                            nc.sync.dma_start(
                                h1s[:],
                                qk_stage[DHEAD:P, 0, ds(col, 512)])
                            nc.vector.tensor_tensor(
                                qkT[DHEAD:P, 0, bi, ds(col, 512)],
                                h1s[:], qbc[:], AluOp.mult)
        psB.close()

        # ================= Phase C/D: attention + out proj =================
        psC = contextlib.ExitStack()
        ps_st_pool = psC.enter_context(
            tc.tile_pool(name="stps", bufs=2, space="PSUM"))
        ps_o_pool = psC.enter_context(
            tc.tile_pool(name="ops", bufs=2, space="PSUM"))

        def attention(bi):
            for qb in range(QB):
                o_ps = [ps_o_pool.tile([DHEAD + 1, 512], F32, tag=f"o{hh}",
                                       name=f"ops{hh}")
                        for hh in range(HLOC)]
                nkt = 4 * (qb + 1)

                def s_pair(kt):
                    d = kt - 4 * qb
                    c0 = max(d, 0) * P
                    sts = []
                    for hh in range(HLOC):
                        st = ps_st_pool.tile([P, 512], F32, tag=f"st{hh}")
                        nc.tensor.matmul(
                            st[:, c0:512],
                            lhsT=qkT[ds(hh * DHEAD, DHEAD), 1, bi,
                                     ts(kt, P)],
                            rhs=qkT[ds(hh * DHEAD, DHEAD), 0, bi,
                                    ds(qb * 512 + c0, 512 - c0)],
                            start=True, stop=True,
                            tile_position=(hh * DHEAD, 0))
                        sts.append(st)
                    return sts

                sts_cur = s_pair(0)
                for kt in range(nkt):
                    sts_next = s_pair(kt + 1) if kt + 1 < nkt else None
                    d = kt - 4 * qb
                    c0 = max(d, 0) * P
                    ets = []
                    for hh in range(HLOC):
                        e_t = e_pool.tile([P, 512], BF16, tag=f"e{hh}")
                        nc.scalar.activation(e_t[:, c0:512],
                                             sts_cur[hh][:, c0:512],
                                             Act.Exp,
                                             scale=rcpk[:, bi, kt,
                                                        hh:hh + 1])
                        if d >= 0:
                            nc.gpsimd.affine_select(
                                out=e_t[:, ds(c0, P)],
                                in_=e_t[:, ds(c0, P)],
                                pattern=[[1, P]],
                                compare_op=AluOp.is_ge,
                                fill=0.0,
                                base=0,
                                channel_multiplier=-1)
                        ets.append(e_t)
                    for hh in range(HLOC):
                        nc.tensor.matmul(o_ps[hh][:, c0:512],
                                         lhsT=v_sb[:, bi, kt, hh, :],
                                         rhs=ets[hh][:, c0:512],
                                         start=(kt == 0),
                                         stop=(kt == nkt - 1))
                    sts_cur = sts_next

                # evac PV psum (bf16) + collect denominator rows
                for hh in range(HLOC):
                    nc.vector.tensor_copy(oU_all[:, bi, qb, hh, :],
                                          o_ps[hh][:])
                    r = qb * HLOC + hh
                    nc.sync.dma_start(
                        pack_o[r:r + 1, bi, :],
                        oU_all[DHEAD:DHEAD + 1, bi, qb, hh, :])
            # batched softmax normalization for this batch
            with nc.allow_low_precision("softmax denom fits bf16"):
                nc.vector.reciprocal(pack_o[:, bi, :], pack_o[:, bi, :])
            for qb in range(QB):
                for hh in range(HLOC):
                    r = qb * HLOC + hh
                    row0 = small.tile([1, 512], BF16, tag="row0")
                    nc.sync.dma_start(row0[:], pack_o[r:r + 1, bi, :])
                    obc = norm_pool.tile([DHEAD, 512], BF16, tag="obc")
                    nc.gpsimd.partition_broadcast(obc[:], row0[:])
                    nc.vector.tensor_tensor(
                        oT[ds(hh * DHEAD, DHEAD), bi, ds(qb * 512, 512)],
                        oU_all[0:DHEAD, bi, qb, hh, :], obc[:],
                        AluOp.mult)

        def d_comm(bi):
            nc.scalar.dma_start(
                cc_in[bi][:].rearrange("s p f -> p s f"),
                oT[:, bi, :].rearrange("p (s f) -> p s f", f=TOK_HALF))
            nc.gpsimd.collective_compute(
                "AllToAll", AluOp.bypass,
                replica_groups=[list(range(NCORES))],
                ins=[cc_in[bi].opt()], outs=[cc_out[bi].opt()])
            nc.scalar.dma_start(oT_all[:, :, bi, :],
                              cc_out[bi][:].rearrange("s p f -> p s f"))

        def d_mm(bi):
            for tt in range(TOK_HALF // P):
                for half in range(2):
                    ps = ps_st_pool.tile([P, 512], F32, tag="st0",
                                         name="outps")
                    for o in range(8):
                        nc.tensor.matmul(
                            ps[:], lhsT=oT_all[:, o, bi, ts(tt, P)],
                            rhs=w_out_sb[:, o, ds(half * 512, 512)],
                            start=(o == 0), stop=(o == 7))
                    ot = out_pool.tile([P, 512], F32, tag="ot")
                    nc.vector.tensor_copy(ot[:], ps[:])
                    nc.sync.dma_start(
                        y_out.ap()[bi, ts(tt, P), ds(half * 512, 512)],
                        ot[:])

        attention(0)
        d_comm(0)
        attention(1)
        d_mm(0)
        d_comm(1)
        d_mm(1)
        if DEBUG_DUMP:
            nc.sync.dma_start(_DBG["dbg_mu"].ap(), mu_all[:])
            nc.sync.dma_start(_DBG["dbg_mudiag"].ap(), mudiag[:])
            nc.sync.dma_start(_DBG["dbg_rstd"].ap(), rstd_all[:])
            nc.sync.dma_start(_DBG["dbg_qkT"].ap(), qkT[:])
            nc.sync.dma_start(_DBG["dbg_vsb"].ap(), v_sb[:])
            nc.sync.dma_start(_DBG["dbg_oT"].ap(), oT[:])
            nc.sync.dma_start(_DBG["dbg_xT"].ap(), xT_sb[:])
            nc.sync.dma_start(_DBG["dbg_qks"].ap(), qk_stage[:])
        psC.close()


# ----------------------------------------------------------------------
# Host side
# ----------------------------------------------------------------------

def make_in_maps(x, ln_w, ln_b, W_qkv, W_out):
    """Build the per-core input maps (host-side sharding/marshaling)."""
    import ml_dtypes
    x = np.asarray(x, dtype=np.float32)
    ln_w = np.asarray(ln_w, dtype=np.float32)
    ln_b = np.asarray(ln_b, dtype=np.float32)
    W_qkv = np.asarray(W_qkv, dtype=np.float32)
    W_out = np.asarray(W_out, dtype=np.float32)

    assert np.allclose(ln_b, 0.0), \
        "kernel folds ln_b@W into a bias; nonzero ln_b not wired up"

    x2 = np.ascontiguousarray(x.reshape(NTOK, DIM))
    x_t = x2.astype(ml_dtypes.bfloat16)
    x_T = np.ascontiguousarray(x_t.T)

    w_eff = (ln_w[:, None] * W_qkv)  # [1024, 3072]
    q_w = w_eff[:, 0 * INNER:1 * INNER]
    k_w = w_eff[:, 1 * INNER:2 * INNER]
    v_w = w_eff[:, 2 * INNER:3 * INNER]
    w_out_bf = W_out.astype(ml_dtypes.bfloat16)

    eye = np.eye(P, dtype=np.float32)
    ones2 = np.zeros((P, 2), dtype=ml_dtypes.bfloat16)
    ones2[0:DHEAD, 0] = 1.0
    ones2[DHEAD:P, 1] = 1.0

    in_maps = []
    for c in range(NCORES):
        h0 = 2 * c
        cols = slice(h0 * DHEAD, (h0 + 2) * DHEAD)
        W3 = np.stack([q_w[:, cols], k_w[:, cols], v_w[:, cols]], axis=0)
        w3b = W3.astype(ml_dtypes.bfloat16)          # [3, 1024, 128]
        w_flip = np.ascontiguousarray(
            w3b.reshape(3, 8, P, P).transpose(2, 0, 1, 3))  # [p, mg, k, m]
        negcs = -w3b.astype(np.float32).sum(axis=1)  # [3, 128]
        negcs8 = np.ascontiguousarray(
            np.broadcast_to(negcs[None], (8, 3, P))).astype(
                ml_dtypes.bfloat16)
        in_maps.append({
            "x_t": x_t,
            "x_T": x_T,
            "w_flip": w_flip,
            "negcs8": negcs8,
            "ones2": ones2,
            "eye": eye,
            "w_out": w_out_bf,
        })
    return in_maps


def gather_output(results):
    """results: list of per-core {name: array} -> full [2, 2048, 1024]."""
    full = np.empty((B, N, DIM), dtype=np.float32)
    for c in range(NCORES):
        part = results[c]["y_out"]  # [B, TOK_HALF, DIM]
        full[:, c * TOK_HALF:(c + 1) * TOK_HALF, :] = part
    return full


_NC_CACHE = None


def kernel(x, ln_w, ln_b, W_qkv, W_out):
    global _NC_CACHE
    from concourse.bass_utils import run_bass_kernel_spmd
    if _NC_CACHE is None:
        _NC_CACHE = build_kernel()
    in_maps = make_in_maps(x, ln_w, ln_b, W_qkv, W_out)
    res = run_bass_kernel_spmd(_NC_CACHE, in_maps,
                               core_ids=list(range(NCORES)))
    return gather_output(res.results)


# revision 30
# speedup vs baseline: 1.0265x; 1.0265x over previous
"""Trainium2 Bass kernel for causal cosine-sim attention block (8 cores), v2.

Reference computation:
  x [2, 2048, 1024] fp32
  xn = LayerNorm(x) * ln_w + ln_b
  qkv = xn @ W_qkv  -> q, k, v   (16 heads x 64)
  q, k l2-normalized per head-dim; sim = (q.k) * 8, causal mask, softmax
  o = attn @ v ; out = o @ W_out   [2, 2048, 1024] fp32

Sharding (8 cores): head-parallel QKV+attention (core c owns heads 2c,2c+1),
token-parallel out projection after per-batch AllToAll.

v2 design (vs v1's 460us):
  - QKV matmuls in flipped orientation psum[cols, tokens]: weights are the
    stationary operand, so each (batch, m-group, half) pass issues only
    8 LDWEIGHTS and streams 1024-token blocks.  The outputs land directly
    in the q^T/k^T [dhead, token] layout attention needs -- no x or qk
    DMA transposes at all.
  - LN mean correction as a K=8 matmul: psum += negcs (x) mu using a
    block-diagonal mu tile (built from a padded [128,128] DMA transpose
    of the per-token means).
  - q/k squared norms via a ones-block matmul (PE) instead of DVE reduce.
  - rstd folds into V after small [64,128] DMA transposes of v^T.
  - LN stats: batch 0 on DVE (bn_stats), batch 1 on ACT via
    activation(..., accum_out=) Copy/Square passes, so neither engine's
    in-order queue stalls the QKV evacuation chains.
  - Attention inner loop software-pipelined: S^T(kt+1) issued before
    PV(kt), so the ACT exp latency never idles the PE (keeps the PE
    p-state ramp at full clock).
  - AllToAll for batch 0 issued before batch 1's attention; both
    out-projections follow all attention matmuls.
  - softmax normalization reads the PV psum directly (no oU evac copy).
"""

import numpy as np

import concourse.bass as bass
import concourse.mybir as mybir
import concourse.tile as tile
from concourse import bacc
from concourse.bass import ts, ds

F32 = mybir.dt.float32
BF16 = mybir.dt.bfloat16

NCORES = 8
DIM = 1024
HEADS = 16
DHEAD = 64
INNER = HEADS * DHEAD          # 1024
B = 2
N = 2048
NTOK = B * N                   # 4096
TOK_HALF = N // NCORES         # 256 tokens per core per batch
HLOC = HEADS // NCORES         # 2 heads per core
P = 128
KT = N // P                    # 16 token tiles per batch
QB = N // 512                  # 4 q-blocks (512) per batch
EPS = 1e-5
SCALE = 8.0
DEBUG_DUMP = False
_DBG = {}
AluOp = mybir.AluOpType
Act = mybir.ActivationFunctionType


def build_kernel():
    nc = bacc.Bacc("TRN2", target_bir_lowering=False, debug=False,
                   num_devices=NCORES)

    x_t = nc.dram_tensor("x_t", [NTOK, DIM], BF16, kind="ExternalInput")
    x_T = nc.dram_tensor("x_T", [DIM, NTOK], BF16, kind="ExternalInput")
    w_flip = nc.dram_tensor("w_flip", [P, 3, 8, P], BF16,
                            kind="ExternalInput")
    negcs8 = nc.dram_tensor("negcs8", [8, 3, P], BF16, kind="ExternalInput")
    ones2 = nc.dram_tensor("ones2", [P, 2], BF16, kind="ExternalInput")
    eye = nc.dram_tensor("eye", [P, P], F32, kind="ExternalInput")
    w_out = nc.dram_tensor("w_out", [INNER, DIM], BF16, kind="ExternalInput")
    y_out = nc.dram_tensor("y_out", [B, TOK_HALF, DIM], F32,
                           kind="ExternalOutput")
    if DEBUG_DUMP:
        global _DBG
        _DBG = {
            "dbg_mu": nc.dram_tensor("dbg_mu", [P, P], BF16,
                                     kind="ExternalOutput"),
            "dbg_mudiag": nc.dram_tensor("dbg_mudiag", [4, B, 2, 2, 512],
                                         BF16, kind="ExternalOutput"),
            "dbg_rstd": nc.dram_tensor("dbg_rstd", [P, B, KT], F32,
                                       kind="ExternalOutput"),
            "dbg_qkT": nc.dram_tensor("dbg_qkT", [P, 2, B, N], BF16,
                                      kind="ExternalOutput"),
            "dbg_vsb": nc.dram_tensor("dbg_vsb", [P, B, KT, HLOC, DHEAD + 1],
                                      BF16, kind="ExternalOutput"),
            "dbg_oT": nc.dram_tensor("dbg_oT", [P, B, N], BF16,
                                     kind="ExternalOutput"),
            "dbg_xT": nc.dram_tensor("dbg_xT", [P, 8, NTOK], BF16,
                                     kind="ExternalOutput"),
            "dbg_qks": nc.dram_tensor("dbg_qks", [P, 1, N], BF16,
                                      kind="ExternalOutput"),
        }

    with tile.TileContext(nc) as tc:
        _body(nc, tc, x_t, x_T, w_flip, negcs8, ones2, eye, w_out, y_out)
    nc.compile()
    return nc


def _body(nc, tc, x_t, x_T, w_flip, negcs8, ones2, eye, w_out, y_out):
    import contextlib
    ctx = contextlib.ExitStack()
    with ctx:
        persist = ctx.enter_context(tc.tile_pool(name="persist", bufs=1))
        xrow_pool = ctx.enter_context(tc.tile_pool(name="xrow", bufs=2))
        small = ctx.enter_context(tc.tile_pool(name="small", bufs=2))
        evac = ctx.enter_context(tc.tile_pool(name="evac", bufs=2))
        vtmp_pool = ctx.enter_context(tc.tile_pool(name="vtmp", bufs=1))
        e_pool = ctx.enter_context(tc.tile_pool(name="epool", bufs=2))
        norm_pool = ctx.enter_context(tc.tile_pool(name="norm", bufs=2))
        out_pool = ctx.enter_context(tc.tile_pool(name="outp", bufs=2))
        dram = ctx.enter_context(tc.tile_pool(name="dram", bufs=1,
                                              space="DRAM"))

        # ---- persistent SBUF (per-partition bytes noted) ----
        xT_sb = persist.tile([P, 8, NTOK], BF16)            # 64 KB
        w_flip_sb = persist.tile([P, 3, 8, P], BF16)        # 6 KB
        negcs_sb = persist.tile([8, 3, P], BF16)
        ones2_sb = persist.tile([P, 2], BF16)
        eye_sb = persist.tile([P, P], F32)
        w_out_sb = persist.tile([P, 8, DIM], BF16)          # 16 KB
        qk_stage = persist.tile([P, 1, N], BF16)            # 4 KB
        qkT = persist.tile([P, 2, B, N], BF16)              # 16 KB
        v_sb = persist.tile([P, B, KT, HLOC, DHEAD + 1], BF16)  # 8.3 KB
        oT = persist.tile([P, B, N], BF16)                  # 8 KB
        oT_all = persist.tile([P, 8, B, TOK_HALF], BF16)    # 8 KB
        mu_all = persist.tile([P, P], BF16)                 # col = bi*16+ti
        muT = {}
        for bi in range(B):
            for half in range(2):
                muT[(bi, half)] = persist.tile([P, P], BF16,
                                               name=f"muT{bi}{half}")
        mudiag = persist.tile([4, B, 2, 2, 512], BF16)      # 8 KB
        mu_f = persist.tile([P, B, KT], F32)
        msq_all = persist.tile([P, B, KT], F32)
        var_all = persist.tile([P, B, KT], F32)
        rstd_all = persist.tile([P, B, KT], F32)
        scr = persist.tile([P, DIM], BF16)                  # ACT scratch
        rcpk = persist.tile([P, B, KT, HLOC], F32)          # 8/|k| columns
        oU_all = persist.tile([DHEAD + 1, B, QB, HLOC, 512], BF16)  # 8KB
        pack_o = persist.tile([8, B, 512], BF16)
        pack_q = persist.tile([8, B, 512], F32)
        eps_t = persist.tile([P, 1], F32)

        nc.scalar.dma_start(w_flip_sb[:], w_flip.ap())
        nc.scalar.dma_start(
            w_out_sb[:], w_out.ap().rearrange("(o p) c -> p o c", p=P))
        nc.sync.dma_start(negcs_sb[:], negcs8.ap())
        nc.sync.dma_start(ones2_sb[:], ones2.ap())
        nc.sync.dma_start(eye_sb[:], eye.ap())
        nc.vector.memset(eps_t[:], EPS)
        nc.vector.memset(mu_all[:], 0.0)
        nc.vector.memset(mudiag[:], 0.0)
        nc.vector.memset(v_sb[:, :, :, :, DHEAD], 1.0)

        # x^T loads on the scalar queue, batch 0 first
        xTr = x_T.ap().rearrange("(o p) t -> p o t", p=P)
        for bi in range(B):
            for k in range(8):
                nc.scalar.dma_start(xT_sb[:, k, ds(bi * N, N)],
                                    xTr[:, k, ds(bi * N, N)])

        cc_in = []
        cc_out = []
        for bi in range(B):
            cci = dram.tile([NCORES, P, TOK_HALF], BF16, name=f"cci{bi}")
            cco = dram.tile([NCORES, P, TOK_HALF], BF16, name=f"cco{bi}")
            cc_in.append(cci)
            cc_out.append(cco)

        # ================= Phase A: LN stats =================
        def stats_tail(bi, ti):
            # mu (bf16, for the transpose) and var = msq - mu^2
            nc.vector.tensor_copy(mu_all[:, bi * KT + ti:bi * KT + ti + 1],
                                  mu_f[:, bi, ti:ti + 1])
            musq = small.tile([P, 1], F32, tag="musq")
            nc.vector.tensor_tensor(musq[:], mu_f[:, bi, ti:ti + 1],
                                    mu_f[:, bi, ti:ti + 1], AluOp.mult)
            nc.vector.tensor_tensor(var_all[:, bi, ti:ti + 1],
                                    msq_all[:, bi, ti:ti + 1], musq[:],
                                    AluOp.subtract)

        def mu_transpose(bi, half):
            t = muT[(bi, half)]
            nc.sync.dma_start_transpose(t[:], mu_all[:])
            for b2 in range(2):
                for r in range(4):
                    row = bi * KT + half * 8 + b2 * 4 + r
                    nc.sync.dma_start(
                        mudiag[r:r + 1, bi, half, b2, ts(r, P)],
                        t[row:row + 1, 0:P])

        for bi in range(B):
            for ti in range(KT):
                i = bi * KT + ti
                xt = xrow_pool.tile([P, DIM], BF16, tag="xt")
                nc.sync.dma_start(xt[:], x_t.ap()[ts(i, P), :])
                if bi == 0:
                    # DVE path: bn_stats (frees ACT for DMA inits)
                    stats = small.tile([P, 2, 6], F32, tag="stats")
                    nc.vector.bn_stats(stats[:, 0, :], xt[:, 0:512])
                    nc.vector.bn_stats(stats[:, 1, :], xt[:, 512:1024])
                    mv = small.tile([P, 2], F32, tag="mv")
                    nc.vector.bn_aggr(mv[:], stats[:])
                    nc.vector.tensor_copy(var_all[:, bi, ti:ti + 1],
                                          mv[:, 1:2])
                    nc.vector.tensor_copy(
                        mu_all[:, i:i + 1], mv[:, 0:1])
                else:
                    # ACT path: accum_out row sums (runs under phase B PE)
                    nc.scalar.activation(scr[:], xt[:], Act.Copy,
                                         scale=1.0 / DIM,
                                         accum_out=mu_f[:, bi, ti:ti + 1])
                    nc.scalar.activation(scr[:], xt[:], Act.Square,
                                         scale=(1.0 / DIM) ** 0.5,
                                         accum_out=msq_all[:, bi, ti:ti + 1])
                    stats_tail(bi, ti)
                if ti == 7:
                    mu_transpose(bi, 0)
                elif ti == KT - 1:
                    mu_transpose(bi, 1)
            # rstd for this batch: one Sqrt + reciprocal
            sd = small.tile([P, KT], F32, tag="sd")
            nc.scalar.activation(sd[:], var_all[:, bi, :], Act.Sqrt,
                                 bias=eps_t[:])
            nc.vector.reciprocal(rstd_all[:, bi, :], sd[:])

        # ================= Phase B: QKV (flipped) =================
        psB = contextlib.ExitStack()
        ps_qkv_pool = psB.enter_context(
            tc.tile_pool(name="qkvps", bufs=2, space="PSUM"))
        ps_ssq_pool = psB.enter_context(
            tc.tile_pool(name="ssqps", bufs=2, space="PSUM"))
        ps_vt_pool = psB.enter_context(
            tc.tile_pool(name="vtps", bufs=2, space="PSUM"))

        for bi in range(B):
            for mg in (2, 0, 1):
                for half in range(2):
                    tok0 = bi * N + half * 1024
                    ps2 = [ps_qkv_pool.tile([P, 512], F32, tag=f"qkv{b2}",
                                            name=f"qkv{b2}")
                           for b2 in range(2)]
                    for k in range(8):
                        for b2 in range(2):
                            nc.tensor.matmul(
                                ps2[b2][:], lhsT=w_flip_sb[:, mg, k, :],
                                rhs=xT_sb[:, k, ds(tok0 + b2 * 512, 512)],
                                start=(k == 0), stop=False)
                    for b2 in range(2):
                        nc.tensor.matmul(
                            ps2[b2][:], lhsT=negcs_sb[0:4, mg, :],
                            rhs=mudiag[:, bi, half, b2, :],
                            start=False, stop=True)
                    if mg == 2:
                        vtmp = vtmp_pool.tile([P, 1024], F32, tag="vtmp")
                        for b2 in range(2):
                            nc.vector.tensor_copy(
                                vtmp[:, ds(b2 * 512, 512)], ps2[b2][:])
                        for c in range(8):
                            kt = half * 8 + c
                            ps_vt = ps_vt_pool.tile([P, P], F32, tag="vT")
                            nc.tensor.transpose(ps_vt[:], vtmp[:, ts(c, P)],
                                                eye_sb[:])
                            nc.vector.tensor_scalar_mul(
                                v_sb[:, bi, kt, :, 0:DHEAD],
                                ps_vt.rearrange("p (h d) -> p h d", d=DHEAD),
                                rstd_all[:, bi, kt:kt + 1])
                    elif mg == 1:
                        # k: store RAW k; 8/|k| folds into the exp scale
                        for b2 in range(2):
                            col = half * 1024 + b2 * 512
                            nc.vector.tensor_copy(
                                qkT[:, mg, bi, ds(col, 512)], ps2[b2][:])
                            seg = qkT[:, mg, bi, ds(col, 512)]
                            sqt = evac.tile([P, 512], BF16, tag="sqt")
                            nc.vector.tensor_tensor(sqt[:], seg, seg,
                                                    AluOp.mult)
                            ssq = ps_ssq_pool.tile([2, 512], F32,
                                                   tag="ssq")
                            nc.tensor.matmul(ssq[:], lhsT=ones2_sb[:],
                                             rhs=sqt[:], start=True,
                                             stop=True)
                            # |k|/8 rows, then PE-transpose to columns
                            sk = small.tile([2, 512], F32, tag="sk")
                            nc.scalar.activation(sk[:], ssq[:], Act.Sqrt,
                                                 scale=1.0 / 64.0)
                            for cc in range(4):
                                kt = half * 8 + b2 * 4 + cc
                                ps_kt = ps_vt_pool.tile([P, P], F32,
                                                        tag="vT",
                                                        name="pskt")
                                nc.tensor.transpose(
                                    ps_kt[:, 0:2], sk[:, ts(cc, P)],
                                    eye_sb[0:2, 0:2])
                                nc.vector.tensor_copy(
                                    rcpk[:, bi, kt, :], ps_kt[:, 0:2])
                        if half == 1:
                            # batched reciprocal of all |k|/8 for this batch
                            nc.vector.reciprocal(
                                rcpk[:, bi].rearrange("p t h -> p (t h)"),
                                rcpk[:, bi].rearrange("p t h -> p (t h)"))
                    else:
                        for b2 in range(2):
                            nc.vector.tensor_copy(
                                qk_stage[:, 0,
                                         ds(half * 1024 + b2 * 512, 512)],
                                ps2[b2][:])
                        for b2 in range(2):
                            col = half * 1024 + b2 * 512
                            seg = qk_stage[:, 0, ds(col, 512)]
                            sqt = evac.tile([P, 512], BF16, tag="sqt")
                            nc.vector.tensor_tensor(sqt[:], seg, seg,
                                                    AluOp.mult)
                            ssq = ps_ssq_pool.tile([2, 512], F32,
                                                   tag="ssq")
                            nc.tensor.matmul(ssq[:], lhsT=ones2_sb[:],
                                             rhs=sqt[:], start=True,
                                             stop=True)
                            rcp = small.tile([2, 512], F32, tag="rcp")
                            nc.scalar.activation(rcp[:], ssq[:], Act.Sqrt)
                            r0 = (half * 2 + b2) * 2
                            nc.sync.dma_start(pack_q[r0:r0 + 1, bi, :],
                                              rcp[0:1, :])
                            nc.sync.dma_start(pack_q[r0 + 1:r0 + 2, bi, :],
                                              rcp[1:2, :])
            # ---- batched q normalization for this batch ----
            nc.vector.reciprocal(pack_q[:, bi, :], pack_q[:, bi, :])
            for half in range(2):
                for b2 in range(2):
                    col = half * 1024 + b2 * 512
                    r0 = (half * 2 + b2) * 2
                    for h in range(2):
                        rowq = small.tile([1, 512], F32, tag="rowB")
                        nc.sync.dma_start(rowq[:],
                                          pack_q[r0 + h:r0 + h + 1, bi, :])
                        qbc = norm_pool.tile([DHEAD, 512], F32, tag="qbc")
                        nc.gpsimd.partition_broadcast(qbc[:], rowq[:])
                        if h == 0:
                            nc.vector.tensor_tensor(
                                qkT[0:DHEAD, 0, bi, ds(col, 512)],
                                qk_stage[0:DHEAD, 0, ds(col, 512)],
                                qbc[:], AluOp.mult)
                        else:
                            h1s = evac.tile([DHEAD, 512], BF16, tag="h1s")
                            nc.sync.dma_start(
                                h1s[:],
                                qk_stage[DHEAD:P, 0, ds(col, 512)])
                            nc.vector.tensor_tensor(
                                qkT[DHEAD:P, 0, bi, ds(col, 512)],
                                h1s[:], qbc[:], AluOp.mult)
        psB.close()

        # ================= Phase C/D: attention + out proj =================
        psC = contextlib.ExitStack()
        ps_st_pool = psC.enter_context(
            tc.tile_pool(name="stps", bufs=2, space="PSUM"))
        ps_o_pool = psC.enter_context(
            tc.tile_pool(name="ops", bufs=2, space="PSUM"))

        def attention(bi):
            for qb in range(QB):
                o_ps = [ps_o_pool.tile([DHEAD + 1, 512], F32, tag=f"o{hh}",
                                       name=f"ops{hh}")
                        for hh in range(HLOC)]
                nkt = 4 * (qb + 1)

                def s_pair(kt):
                    d = kt - 4 * qb
                    c0 = max(d, 0) * P
                    sts = []
                    for hh in range(HLOC):
                        st = ps_st_pool.tile([P, 512], F32, tag=f"st{hh}")
                        nc.tensor.matmul(
                            st[:, c0:512],
                            lhsT=qkT[ds(hh * DHEAD, DHEAD), 1, bi,
                                     ts(kt, P)],
                            rhs=qkT[ds(hh * DHEAD, DHEAD), 0, bi,
                                    ds(qb * 512 + c0, 512 - c0)],
                            start=True, stop=True,
                            tile_position=(hh * DHEAD, 0))
                        sts.append(st)
                    return sts

                sts_cur = s_pair(0)
                for kt in range(nkt):
                    sts_next = s_pair(kt + 1) if kt + 1 < nkt else None
                    d = kt - 4 * qb
                    c0 = max(d, 0) * P
                    ets = []
                    for hh in range(HLOC):
                        e_t = e_pool.tile([P, 512], BF16, tag=f"e{hh}")
                        nc.scalar.activation(e_t[:, c0:512],
                                             sts_cur[hh][:, c0:512],
                                             Act.Exp,
                                             scale=rcpk[:, bi, kt,
                                                        hh:hh + 1])
                        if d >= 0:
                            nc.gpsimd.affine_select(
                                out=e_t[:, ds(c0, P)],
                                in_=e_t[:, ds(c0, P)],
                                pattern=[[1, P]],
                                compare_op=AluOp.is_ge,
                                fill=0.0,
                                base=0,
                                channel_multiplier=-1)
                        ets.append(e_t)
                    for hh in range(HLOC):
                        nc.tensor.matmul(o_ps[hh][:, c0:512],
                                         lhsT=v_sb[:, bi, kt, hh, :],
                                         rhs=ets[hh][:, c0:512],
                                         start=(kt == 0),
                                         stop=(kt == nkt - 1))
                    sts_cur = sts_next

                # evac PV psum (bf16) + collect denominator rows
                for hh in range(HLOC):
                    nc.vector.tensor_copy(oU_all[:, bi, qb, hh, :],
                                          o_ps[hh][:])
                    r = qb * HLOC + hh
                    nc.sync.dma_start(
                        pack_o[r:r + 1, bi, :],
                        oU_all[DHEAD:DHEAD + 1, bi, qb, hh, :])
            # batched softmax normalization for this batch
            with nc.allow_low_precision("softmax denom fits bf16"):
                nc.vector.reciprocal(pack_o[:, bi, :], pack_o[:, bi, :])
            for qb in range(QB):
                for hh in range(HLOC):
                    r = qb * HLOC + hh
                    row0 = small.tile([1, 512], BF16, tag="row0")
                    nc.sync.dma_start(row0[:], pack_o[r:r + 1, bi, :])
                    obc = norm_pool.tile([DHEAD, 512], BF16, tag="obc")
                    nc.gpsimd.partition_broadcast(obc[:], row0[:])
                    nc.vector.tensor_tensor(
                        oT[ds(hh * DHEAD, DHEAD), bi, ds(qb * 512, 512)],
                        oU_all[0:DHEAD, bi, qb, hh, :], obc[:],
                        AluOp.mult)

        def d_comm(bi):
            nc.sync.dma_start(
                cc_in[bi][:].rearrange("s p f -> p s f"),
                oT[:, bi, :].rearrange("p (s f) -> p s f", f=TOK_HALF))
            nc.gpsimd.collective_compute(
                "AllToAll", AluOp.bypass,
                replica_groups=[list(range(NCORES))],
                ins=[cc_in[bi].opt()], outs=[cc_out[bi].opt()])

        def d_mm(bi):
            nc.sync.dma_start(oT_all[:, :, bi, :],
                              cc_out[bi][:].rearrange("s p f -> p s f"))
            for tt in range(TOK_HALF // P):
                for half in range(2):
                    ps = ps_st_pool.tile([P, 512], F32, tag="st0",
                                         name="outps")
                    for o in range(8):
                        nc.tensor.matmul(
                            ps[:], lhsT=oT_all[:, o, bi, ts(tt, P)],
                            rhs=w_out_sb[:, o, ds(half * 512, 512)],
                            start=(o == 0), stop=(o == 7))
                    ot = out_pool.tile([P, 512], F32, tag="ot")
                    nc.vector.tensor_copy(ot[:], ps[:])
                    nc.sync.dma_start(
                        y_out.ap()[bi, ts(tt, P), ds(half * 512, 512)],
                        ot[:])

        attention(0)
        d_comm(0)
        attention(1)
        d_mm(0)
        d_comm(1)
        d_mm(1)
        if DEBUG_DUMP:
            nc.sync.dma_start(_DBG["dbg_mu"].ap(), mu_all[:])
            nc.sync.dma_start(_DBG["dbg_mudiag"].ap(), mudiag[:])
            nc.sync.dma_start(_DBG["dbg_rstd"].ap(), rstd_all[:])
            nc.sync.dma_start(_DBG["dbg_qkT"].ap(), qkT[:])
            nc.sync.dma_start(_DBG["dbg_vsb"].ap(), v_sb[:])
            nc.sync.dma_start(_DBG["dbg_oT"].ap(), oT[:])
            nc.sync.dma_start(_DBG["dbg_xT"].ap(), xT_sb[:])
            nc.sync.dma_start(_DBG["dbg_qks"].ap(), qk_stage[:])
        psC.close()


# ----------------------------------------------------------------------
# Host side
# ----------------------------------------------------------------------

def make_in_maps(x, ln_w, ln_b, W_qkv, W_out):
    """Build the per-core input maps (host-side sharding/marshaling)."""
    import ml_dtypes
    x = np.asarray(x, dtype=np.float32)
    ln_w = np.asarray(ln_w, dtype=np.float32)
    ln_b = np.asarray(ln_b, dtype=np.float32)
    W_qkv = np.asarray(W_qkv, dtype=np.float32)
    W_out = np.asarray(W_out, dtype=np.float32)

    assert np.allclose(ln_b, 0.0), \
        "kernel folds ln_b@W into a bias; nonzero ln_b not wired up"

    x2 = np.ascontiguousarray(x.reshape(NTOK, DIM))
    x_t = x2.astype(ml_dtypes.bfloat16)
    x_T = np.ascontiguousarray(x_t.T)

    w_eff = (ln_w[:, None] * W_qkv)  # [1024, 3072]
    q_w = w_eff[:, 0 * INNER:1 * INNER]
    k_w = w_eff[:, 1 * INNER:2 * INNER]
    v_w = w_eff[:, 2 * INNER:3 * INNER]
    w_out_bf = W_out.astype(ml_dtypes.bfloat16)

    eye = np.eye(P, dtype=np.float32)
    ones2 = np.zeros((P, 2), dtype=ml_dtypes.bfloat16)
    ones2[0:DHEAD, 0] = 1.0
    ones2[DHEAD:P, 1] = 1.0

    in_maps = []
    for c in range(NCORES):
        h0 = 2 * c
        cols = slice(h0 * DHEAD, (h0 + 2) * DHEAD)
        W3 = np.stack([q_w[:, cols], k_w[:, cols], v_w[:, cols]], axis=0)
        w3b = W3.astype(ml_dtypes.bfloat16)          # [3, 1024, 128]
        w_flip = np.ascontiguousarray(
            w3b.reshape(3, 8, P, P).transpose(2, 0, 1, 3))  # [p, mg, k, m]
        negcs = -w3b.astype(np.float32).sum(axis=1)  # [3, 128]
        negcs8 = np.ascontiguousarray(
            np.broadcast_to(negcs[None], (8, 3, P))).astype(
                ml_dtypes.bfloat16)
        in_maps.append({
            "x_t": x_t,
            "x_T": x_T,
            "w_flip": w_flip,
            "negcs8": negcs8,
            "ones2": ones2,
            "eye": eye,
            "w_out": w_out_bf,
        })
    return in_maps


def gather_output(results):
    """results: list of per-core {name: array} -> full [2, 2048, 1024]."""
    full = np.empty((B, N, DIM), dtype=np.float32)
    for c in range(NCORES):
        part = results[c]["y_out"]  # [B, TOK_HALF, DIM]
        full[:, c * TOK_HALF:(c + 1) * TOK_HALF, :] = part
    return full


_NC_CACHE = None


def kernel(x, ln_w, ln_b, W_qkv, W_out):
    global _NC_CACHE
    from concourse.bass_utils import run_bass_kernel_spmd
    if _NC_CACHE is None:
        _NC_CACHE = build_kernel()
    in_maps = make_in_maps(x, ln_w, ln_b, W_qkv, W_out)
    res = run_bass_kernel_spmd(_NC_CACHE, in_maps,
                               core_ids=list(range(NCORES)))
    return gather_output(res.results)
